# revision 40
# baseline (speedup 1.0000x reference)
"""Trainium2 Bass kernel for nn_AdditiveRecursiveNN (depth-13 binary tree of
64x64 matmuls with per-node weights gathered from a 50000x4096 table).

Sharding: data-parallel over the 16 independent depth-9 subtrees rooted at
heap nodes 15..30 -- TWO subtrees per NeuronCore ("L" rows 0:64, "R" rows
64:128). The host gathers each core's node weights into a dense stream; the
leaf level is folded into the pack (s8 = relu(W_leaf_2t) + relu(W_leaf_2t+1)
is the level-7 matmul rhs), and the device runs half-tree levels 7..3:
h = relu(W @ (h_l + h_r) + b). Level-3 outputs (global heap level 7, all
128 nodes) are shipped out; the host finishes the top 7 levels in f32.

Device scheme per slot (one L/R node pair):
 - weights arrive as a BLOCK-DIAGONAL 128x128 fp8 tile per slot (WL^T in
   rows/cols 0:64, WR^T in rows/cols 64:128, zeros shipped in the stream so
   every DMA is fully contiguous): ONE 128-col LDWEIGHTS + ONE K=128 M=128
   N=64 matmul computes both nodes. Weights are pre-scaled by 256 (fp8e4m3
   precision); drains divide it back out. Activations stay bf16.
 - bias handling uses an OFFSET REFORMULATION so most PSUM banks need no
   rank-1 bias matmul on the (1.2 GHz-pinned, column-rate-bound) PE: the
   device stores h° = h + phi with host-chosen per-node offset MATRICES
   phi. Child sums s~ = h°_l + h°_r then satisfy
     relu(W s + b) + phi = max((W s~)*inv, phi),  phi = W(phi_l + phi_r) - b,
   so the drain is a single DVE scalar_tensor_tensor (mult inv, max beta)
   against a host-packed beta stream, over one or two PSUM banks at a time.
   A block of level-7 banks (whose children come from the host, so phi_c=0
   and the rank-1 K=2 ones-matmul bias is exact) keeps the classic
   PE-bias + ScalarE-relu path to balance PE/ScalarE/DVE load.
"""
import sys
sys.path.insert(0, '/opt/trn_rl_repo')

import numpy as np
import ml_dtypes

E = 64
D = 13
N_NODES = 2 ** D - 1          # 8191
NCORES = 8
HT_D = 9                      # half-tree depth: levels 0..8 (8 = leaves)
DEV_LEVELS = [7, 6, 5, 4, 3]  # half-tree levels computed on device
NSLOT = sum(2 ** l for l in DEV_LEVELS)      # 248 slot-pairs per core
S8_SLOTS = 2 ** 7                            # 128 level-7 rhs slots
OUT_SLOTS = 2 ** 3                           # 8 level-3 outputs per half
NBANK = NSLOT // 8                           # 31 PSUM bank-groups
# ScalarE-path banks must be a child-closed set (their children all SC or
# host-fed) so the rank-1 PE bias stays exact: level-7 banks only (their
# children come from the host leaf stream).
SC_BANKS = frozenset(range(0, 11))
WSCALE = 256.0                # fp8 weight/bias pre-scale
S8SCALE = 64.0                # fp8 leaf-sum pre-scale

# slots drained via the DVE beta path, in stream order -> beta stream column
DVE_SLOTS = [t for t in range(NSLOT) if (t // 8) not in SC_BANKS]
BPOS = {t: i for i, t in enumerate(DVE_SLOTS)}
N_BB7 = sum(1 for t in DVE_SLOTS if t < 128)  # level-7 beta slots (early)

_CACHE = {}


def _drain_units():
    """Per level: list of (slot_start, n_slots, kind) drain groups."""
    units = {}
    base = 0
    for lvl in DEV_LEVELS:
        n = 2 ** lvl
        u = []
        t = 0
        while t < n:
            bank = (base + t) // 8
            if bank in SC_BANKS:
                u.append((t, 8, "sc"))
                t += 8
            elif lvl >= 6 and t + 16 <= n and (bank + 1) not in SC_BANKS:
                # pair adjacent DVE banks into one 2-bank drain (throughput);
                # small levels keep 8-slot units (tail-chain latency)
                u.append((t, 16, "dve"))
                t += 16
            else:
                u.append((t, 8, "dve"))
                t += 8
        units[lvl] = u
        base += n
    return units


def _build_nc():
    import concourse.bacc as bacc
    import concourse.tile as tile
    import concourse.mybir as mybir

    f32 = mybir.dt.float32
    bf16 = mybir.dt.bfloat16
    fp8 = mybir.dt.float8e4
    nc = bacc.Bacc(None, target_bir_lowering=False)

    wt = nc.dram_tensor("wt", [128, NSLOT * 128], fp8, kind="ExternalInput")
    s8 = nc.dram_tensor("s8", [128, S8_SLOTS * E], fp8, kind="ExternalInput")
    bi = nc.dram_tensor("bi", [2, NSLOT * E], bf16, kind="ExternalInput")
    on2 = nc.dram_tensor("on2", [2, 128], bf16, kind="ExternalInput")
    bb = (nc.dram_tensor("bb", [128, len(DVE_SLOTS) * E], fp8,
                         kind="ExternalInput") if DVE_SLOTS else None)
    out = nc.dram_tensor("out", [128, OUT_SLOTS * E], f32, kind="ExternalOutput")

    units = _drain_units()

    with tile.TileContext(nc) as tc:
        with (
            tc.tile_pool(name="cst", bufs=1) as pool_c,
            tc.tile_pool(name="s8p", bufs=1) as pool_s8,
            tc.tile_pool(name="wtp", bufs=4) as pool_wt,
            tc.tile_pool(name="h", bufs=1) as pool_h,
            tc.tile_pool(name="s", bufs=1) as pool_s,
            tc.tile_pool(name="ob", bufs=1) as pool_o,
            tc.tile_pool(name="w0", bufs=1) as pool_w0,
            tc.tile_pool(name="ps1", bufs=2, space="PSUM") as pool_ps1,
            tc.tile_pool(name="ps2", bufs=3, space="PSUM") as pool_ps2,
        ):
            # DMA rings (measured): HWDGE rings (sync/scalar) run ~140GB/s
            # each but the scalar ring starves once ScalarE starts its
            # ACTIVATE drains (~20us); the gpsimd SWDGE ring has a ~6us slow
            # start. So: scalar ring front-loads everything needed early,
            # sync carries the mid weight stream + out, gpsimd the tail.
            bt = pool_c.tile([2, NSLOT * E], bf16)
            nc.scalar.dma_start(bt[:], bi[:])
            ones2 = pool_c.tile([2, 128], bf16)
            nc.scalar.dma_start(ones2[:], on2[:])

            w0a = pool_w0.tile([128, 32 * 128], fp8, tag="w0a")
            nc.sync.dma_start(w0a[:], wt[:, 0:32 * 128])

            s8t = pool_s8.tile([128, S8_SLOTS * E], fp8)
            nc.scalar.dma_start(s8t[:, 0:64 * E], s8[:, 0:64 * E])
            w0b = pool_w0.tile([128, 32 * 128], fp8, tag="w0b")
            nc.scalar.dma_start(w0b[:], wt[:, 32 * 128:64 * 128])
            nc.scalar.dma_start(s8t[:, 64 * E:128 * E], s8[:, 64 * E:128 * E])

            # betas: level-7 part first on gpsimd (needed ~19us), rest later
            bbt = None
            if DVE_SLOTS:
                bbt = pool_c.tile([128, len(DVE_SLOTS) * E], fp8)
                if N_BB7:
                    nc.gpsimd.dma_start(bbt[:, 0:N_BB7 * E], bb[:, 0:N_BB7 * E])

            slot_chunk = [(w0a, t) for t in range(32)]
            slot_chunk += [(w0b, t) for t in range(32)]
            chunk_spec = [(64, 64, nc.sync), (128, 64, nc.sync),
                          (192, 56, nc.gpsimd)]
            for (base, n, eng) in chunk_spec:
                w = pool_wt.tile([128, 64 * 128], fp8, tag="wt")
                eng.dma_start(
                    w[:, 0:n * 128], wt[:, base * 128:(base + n) * 128])
                slot_chunk += [(w, t) for t in range(n)]

            # late betas (levels 6..3) on gpsimd
            if DVE_SLOTS and len(DVE_SLOTS) > N_BB7:
                nc.gpsimd.dma_start(bbt[:, N_BB7 * E:], bb[:, N_BB7 * E:])

            slot_base = 0
            s_cur = s8t
            h_prev = None
            for lvl in DEV_LEVELS:
                n = 2 ** lvl
                inv = 1.0 / (WSCALE * S8SCALE) if lvl == 7 else 1.0 / WSCALE
                if lvl > 3:
                    h_new = pool_h.tile([128, n * E], bf16, tag=f"h{lvl}")
                else:
                    h_new = pool_o.tile([128, n * E], f32, tag="hout")

                if lvl < 7:
                    s_cur = pool_s.tile([128, n * E], bf16, tag=f"s{lvl}")
                    for t0 in range(0, n, 16):
                        tn = min(16, n - t0)
                        pairs = h_prev[:, 2 * t0 * E:2 * (t0 + tn) * E].rearrange(
                            "p (t c) -> p t c", c=2 * E)
                        nc.vector.tensor_add(
                            s_cur[:, t0 * E:(t0 + tn) * E].rearrange(
                                "p (t m) -> p t m", m=E),
                            pairs[:, :, 0:E], pairs[:, :, E:2 * E])

                for (g0, gn, kind) in units[lvl]:
                    pool = pool_ps1 if gn == 8 else pool_ps2
                    ps = pool.tile([128, gn * E], f32, tag=f"ps{gn}")
                    if kind == "sc":
                        boff = (slot_base + g0) * E
                        nc.tensor.matmul(
                            out=ps[:, :], lhsT=ones2[:, :],
                            rhs=bt[:, boff:boff + gn * E], start=True,
                            stop=False, skip_group_check=True)
                    for i in range(gn):
                        t = g0 + i
                        w, off = slot_chunk[slot_base + t]
                        nc.tensor.matmul(
                            out=ps[:, i * E:(i + 1) * E],
                            lhsT=w[:, off * 128:(off + 1) * 128],
                            rhs=s_cur[:, t * E:(t + 1) * E],
                            start=(kind == "dve" and i % 8 == 0), stop=True,
                            skip_group_check=True)
                    dst = h_new[:, g0 * E:(g0 + gn) * E]
                    if kind == "sc":
                        # alternate drain engine per bank: bias is already in
                        # PSUM, so DVE can relu via a two-op tensor_scalar
                        if lvl == 3 or ((slot_base + g0) // 8) % 2 == 0:
                            nc.scalar.activation(
                                out=dst, in_=ps[:, :],
                                func=mybir.ActivationFunctionType.Relu,
                                scale=inv)
                        else:
                            nc.vector.tensor_scalar(
                                dst, ps[:, :], inv, 0.0,
                                op0=mybir.AluOpType.mult,
                                op1=mybir.AluOpType.max)
                    else:
                        do = BPOS[slot_base + g0] * E
                        nc.vector.scalar_tensor_tensor(
                            dst, ps[:, :], inv, bbt[:, do:do + gn * E],
                            op0=mybir.AluOpType.mult, op1=mybir.AluOpType.max)

                h_prev = h_new
                slot_base += n

            nc.sync.dma_start(out[:], h_prev[:, :])

    nc.compile()
    return nc


def _get_nc():
    if "nc" not in _CACHE:
        _CACHE["nc"] = _build_nc()
    return _CACHE["nc"]


def _pack_core(c, node_ids, emb, bias_table):
    """Packed wt/s8/bi/bb streams for core c (half-trees rooted at heap
    nodes 15+2c and 16+2c). Returns (in_map, phi3[2,8,E,E])."""
    wtz = np.zeros((2, E, NSLOT, 128), dtype=np.float32)  # [half, j, slot, col]
    bi = np.empty((2, NSLOT, E), dtype=np.float32)
    phiv = np.zeros((2, NSLOT, E, E), dtype=np.float32)   # per-node offsets
    s8p = np.empty((2, E, S8_SLOTS, E), dtype=np.float32)
    level_base = {}
    base = 0
    for lvl in DEV_LEVELS:
        level_base[lvl] = base
        base += 2 ** lvl
    for q in range(2):
        g0 = 15 + 2 * c + q
        Wlv, blv = {}, {}
        for lvl in DEV_LEVELS:
            n = 2 ** lvl
            start = (g0 + 1) * n - 1
            ids = node_ids[start:start + n]
            Wlv[lvl] = emb[ids].reshape(n, E, E)
            blv[lvl] = bias_table[ids].astype(np.float32)
            woff = level_base[lvl]
            wtz[q, :, woff:woff + n, q * E:(q + 1) * E] = \
                (Wlv[lvl] * WSCALE).transpose(2, 0, 1)
        # phi recursion in stream order 7 -> 3 (children first)
        phi = {}
        for lvl in DEV_LEVELS:
            n = 2 ** lvl
            woff = level_base[lvl]
            if lvl == 7:
                g = np.zeros((n, E, E), dtype=np.float32)
            else:
                pc = phi[lvl + 1]
                g = np.einsum('nij,njk->nik', Wlv[lvl], pc[0::2] + pc[1::2])
            bsc = WSCALE * S8SCALE if lvl == 7 else WSCALE
            ph = np.zeros((n, E, E), dtype=np.float32)
            for t in range(n):
                if (woff + t) // 8 not in SC_BANKS:
                    ph[t] = g[t] - blv[lvl][t][None, :]
            phi[lvl] = ph
            bi[q, woff:woff + n, :] = blv[lvl] * bsc  # used by SC banks (g=0)
            phiv[q, woff:woff + n] = ph
        # leaf level folded on host: s8 slot t = relu(W_2t) + relu(W_2t+1)
        nleaf = 2 ** (HT_D - 1)
        start = (g0 + 1) * nleaf - 1
        ids = node_ids[start:start + nleaf]
        leaf = np.maximum(emb[ids].reshape(nleaf, E, E), 0.0)
        s8p[q] = (leaf[0::2] + leaf[1::2]).transpose(1, 0, 2) * S8SCALE
    # beta stream: [128 partitions(i per half), dve-slot, k]
    bbs = np.empty((2, E, len(DVE_SLOTS), E), dtype=np.float32)
    for t in DVE_SLOTS:
        p = BPOS[t]
        for q in range(2):
            bbs[q, :, p, :] = phiv[q, t]
    on2 = np.zeros((2, 128), dtype=np.float32)
    on2[0, 0:E] = 1.0
    on2[1, E:128] = 1.0
    phi3 = phiv[:, level_base[3]:level_base[3] + 8]  # [2, 8, E, E]
    return {
        "wt": np.ascontiguousarray(wtz.reshape(128, NSLOT * 128)).astype(ml_dtypes.float8_e4m3),
        "s8": np.ascontiguousarray(s8p.reshape(128, S8_SLOTS * E)).astype(ml_dtypes.float8_e4m3),
        "bi": np.ascontiguousarray(bi.reshape(2, NSLOT * E)).astype(ml_dtypes.bfloat16),
        "on2": on2.astype(ml_dtypes.bfloat16),
        **({"bb": np.ascontiguousarray(bbs.reshape(128, len(DVE_SLOTS) * E)).astype(ml_dtypes.float8_e4m3)} if DVE_SLOTS else {}),
    }, phi3


def kernel(node_ids, label, embedding, bias_table, proj_w, proj_b):
    from concourse.bass_utils import run_bass_kernel_spmd

    node_ids = np.asarray(node_ids).astype(np.int64)
    emb = np.ascontiguousarray(np.asarray(embedding, dtype=np.float32))
    bias_table = np.ascontiguousarray(np.asarray(bias_table, dtype=np.float32))
    proj_w = np.asarray(proj_w, dtype=np.float32)
    proj_b = np.asarray(proj_b, dtype=np.float32)
    label_i = int(np.asarray(label))

    nc = _get_nc()
    packs = [_pack_core(c, node_ids, emb, bias_table) for c in range(NCORES)]
    in_maps = [p[0] for p in packs]
    res = run_bass_kernel_spmd(nc, in_maps, core_ids=list(range(NCORES)))

    # device ships h° at global heap level 7 (nodes 127..254): h = h° - phi
    h = np.empty((128, E, E), dtype=np.float32)
    for c in range(NCORES):
        o = res.results[c]["out"].astype(np.float32)  # [128, 8*64]
        phi3 = packs[c][1]
        for q in range(2):
            g0 = 15 + 2 * c + q
            base = (g0 + 1) * OUT_SLOTS - 1 - 127
            for t in range(OUT_SLOTS):
                h[base + t] = o[q * E:(q + 1) * E, t * E:(t + 1) * E] \
                    - phi3[q, t]
    for lvl in range(6, -1, -1):
        nlv = 2 ** lvl
        start = nlv - 1
        ids = node_ids[start:start + nlv]
        W = emb[ids].reshape(nlv, E, E)
        b = bias_table[ids]
        s = h[0::2] + h[1::2]
        h = np.maximum(np.einsum('nij,njk->nik', W, s) + b[:, None, :], 0.0)

    root = h[0].reshape(-1)
    logits = root @ proj_w.T + proj_b
    m = logits.max()
    lse = m + np.log(np.exp(logits - m).sum())
    log_softmax = logits - lse
    loss = np.float32(-log_softmax[label_i])
    prediction = np.int64(np.argmax(logits))
    return prediction, loss


# revision 41
# speedup vs baseline: 1.0955x; 1.0955x over previous
"""Trainium2 Bass kernel for nn_AdditiveRecursiveNN (depth-13 binary tree of
64x64 matmuls with per-node weights gathered from a 50000x4096 table).

Sharding: data-parallel over the 16 independent depth-9 subtrees rooted at
heap nodes 15..30 -- TWO subtrees per NeuronCore ("L" rows 0:64, "R" rows
64:128). The host gathers each core's node weights into a dense stream; the
leaf level is folded into the pack (s8 = relu(W_leaf_2t) + relu(W_leaf_2t+1)
is the level-7 matmul rhs), and the device runs half-tree levels 7..3:
h = relu(W @ (h_l + h_r) + b). Level-3 outputs (global heap level 7, all
128 nodes) are shipped out; the host finishes the top 7 levels in f32.

Device scheme per slot (one L/R node pair):
 - weights arrive as a BLOCK-DIAGONAL 128x128 fp8 tile per slot (WL^T in
   rows/cols 0:64, WR^T in rows/cols 64:128, zeros shipped in the stream so
   every DMA is fully contiguous): ONE 128-col LDWEIGHTS + ONE K=128 M=128
   N=64 matmul computes both nodes. Weights are pre-scaled by 256 (fp8e4m3
   precision); drains divide it back out. Activations stay bf16.
 - bias handling uses an OFFSET REFORMULATION so most PSUM banks need no
   rank-1 bias matmul on the (1.2 GHz-pinned, column-rate-bound) PE: the
   device stores h° = h + phi with host-chosen per-node offset MATRICES
   phi. Child sums s~ = h°_l + h°_r then satisfy
     relu(W s + b) + phi = max((W s~)*inv, phi),  phi = W(phi_l + phi_r) - b,
   so the drain is a single DVE scalar_tensor_tensor (mult inv, max beta)
   against a host-packed beta stream, over one or two PSUM banks at a time.
   A block of level-7 banks (whose children come from the host, so phi_c=0
   and the rank-1 K=2 ones-matmul bias is exact) keeps the classic
   PE-bias + ScalarE-relu path to balance PE/ScalarE/DVE load.
"""
import sys
sys.path.insert(0, '/opt/trn_rl_repo')

import numpy as np
import ml_dtypes

E = 64
D = 13
N_NODES = 2 ** D - 1          # 8191
NCORES = 8
HT_D = 9                      # half-tree depth: levels 0..8 (8 = leaves)
DEV_LEVELS = [7, 6, 5, 4, 3]  # half-tree levels computed on device
NSLOT = sum(2 ** l for l in DEV_LEVELS)      # 248 slot-pairs per core
S8_SLOTS = 2 ** 7                            # 128 level-7 rhs slots
OUT_SLOTS = 2 ** 3                           # 8 level-3 outputs per half
NBANK = NSLOT // 8                           # 31 PSUM bank-groups
# ScalarE-path banks must be a child-closed set (their children all SC or
# host-fed) so the rank-1 PE bias stays exact. All-SC = classic PE-bias
# everywhere; the PE is then the dense self-pacing metronome (measured best).
SC_BANKS = frozenset(range(NBANK))
WSCALE = 256.0                # fp8 weight/bias pre-scale
S8SCALE = 64.0                # fp8 leaf-sum pre-scale

# slots drained via the DVE beta path, in stream order -> beta stream column
DVE_SLOTS = [t for t in range(NSLOT) if (t // 8) not in SC_BANKS]
BPOS = {t: i for i, t in enumerate(DVE_SLOTS)}
N_BB7 = sum(1 for t in DVE_SLOTS if t < 128)  # level-7 beta slots (early)

_CACHE = {}


def _drain_units():
    """Per level: list of (slot_start, n_slots, kind) drain groups."""
    units = {}
    base = 0
    for lvl in DEV_LEVELS:
        n = 2 ** lvl
        u = []
        t = 0
        while t < n:
            bank = (base + t) // 8
            if bank in SC_BANKS:
                u.append((t, 8, "sc"))
                t += 8
            elif t + 16 <= n and (bank + 1) not in SC_BANKS:
                # pair adjacent DVE banks into one 2-bank drain
                u.append((t, 16, "dve"))
                t += 16
            else:
                u.append((t, 8, "dve"))
                t += 8
        units[lvl] = u
        base += n
    return units


def _build_nc():
    import concourse.bacc as bacc
    import concourse.tile as tile
    import concourse.mybir as mybir

    f32 = mybir.dt.float32
    bf16 = mybir.dt.bfloat16
    fp8 = mybir.dt.float8e4
    nc = bacc.Bacc(None, target_bir_lowering=False)

    wt = nc.dram_tensor("wt", [128, NSLOT * 128], fp8, kind="ExternalInput")
    s8 = nc.dram_tensor("s8", [128, S8_SLOTS * E], fp8, kind="ExternalInput")
    bi = nc.dram_tensor("bi", [2, NSLOT * E], bf16, kind="ExternalInput")
    on2 = nc.dram_tensor("on2", [2, 128], bf16, kind="ExternalInput")
    bb = (nc.dram_tensor("bb", [128, len(DVE_SLOTS) * E], fp8,
                         kind="ExternalInput") if DVE_SLOTS else None)
    out = nc.dram_tensor("out", [128, OUT_SLOTS * E], f32, kind="ExternalOutput")

    units = _drain_units()

    with tile.TileContext(nc) as tc:
        with (
            tc.tile_pool(name="cst", bufs=1) as pool_c,
            tc.tile_pool(name="s8p", bufs=1) as pool_s8,
            tc.tile_pool(name="wtp", bufs=4) as pool_wt,
            tc.tile_pool(name="h", bufs=1) as pool_h,
            tc.tile_pool(name="s", bufs=1) as pool_s,
            tc.tile_pool(name="ob", bufs=1) as pool_o,
            tc.tile_pool(name="w0", bufs=1) as pool_w0,
            tc.tile_pool(name="ps1", bufs=2, space="PSUM") as pool_ps1,
            tc.tile_pool(name="ps2", bufs=3, space="PSUM") as pool_ps2,
        ):
            # DMA rings (measured): HWDGE rings (sync/scalar) run ~140GB/s
            # each but the scalar ring starves once ScalarE starts its
            # ACTIVATE drains (~20us); the gpsimd SWDGE ring has a ~6us slow
            # start. So: scalar ring front-loads everything needed early,
            # sync carries the mid weight stream + out, gpsimd the tail.
            bt = pool_c.tile([2, NSLOT * E], bf16)
            nc.scalar.dma_start(bt[:], bi[:])
            ones2 = pool_c.tile([2, 128], bf16)
            nc.scalar.dma_start(ones2[:], on2[:])

            w0a = pool_w0.tile([128, 32 * 128], fp8, tag="w0a")
            nc.sync.dma_start(w0a[:], wt[:, 0:32 * 128])

            s8t = pool_s8.tile([128, S8_SLOTS * E], fp8)
            nc.scalar.dma_start(s8t[:, 0:64 * E], s8[:, 0:64 * E])
            w0b = pool_w0.tile([128, 32 * 128], fp8, tag="w0b")
            nc.scalar.dma_start(w0b[:], wt[:, 32 * 128:64 * 128])
            nc.scalar.dma_start(s8t[:, 64 * E:128 * E], s8[:, 64 * E:128 * E])

            # betas: level-7 part first on gpsimd (needed ~19us), rest later
            bbt = None
            if DVE_SLOTS:
                bbt = pool_c.tile([128, len(DVE_SLOTS) * E], fp8)
                if N_BB7:
                    nc.gpsimd.dma_start(bbt[:, 0:N_BB7 * E], bb[:, 0:N_BB7 * E])

            slot_chunk = [(w0a, t) for t in range(32)]
            slot_chunk += [(w0b, t) for t in range(32)]
            chunk_spec = [(64, 64, nc.sync), (128, 64, nc.sync),
                          (192, 56, nc.gpsimd)]
            for (base, n, eng) in chunk_spec:
                w = pool_wt.tile([128, 64 * 128], fp8, tag="wt")
                eng.dma_start(
                    w[:, 0:n * 128], wt[:, base * 128:(base + n) * 128])
                slot_chunk += [(w, t) for t in range(n)]

            # late betas (levels 6..3) on gpsimd
            if DVE_SLOTS and len(DVE_SLOTS) > N_BB7:
                nc.gpsimd.dma_start(bbt[:, N_BB7 * E:], bb[:, N_BB7 * E:])

            slot_base = 0
            s_cur = s8t
            h_prev = None
            for lvl in DEV_LEVELS:
                n = 2 ** lvl
                inv = 1.0 / (WSCALE * S8SCALE) if lvl == 7 else 1.0 / WSCALE
                if lvl > 3:
                    h_new = pool_h.tile([128, n * E], bf16, tag=f"h{lvl}")
                else:
                    h_new = pool_o.tile([128, n * E], f32, tag="hout")

                if lvl < 7:
                    s_cur = pool_s.tile([128, n * E], bf16, tag=f"s{lvl}")
                    for t0 in range(0, n, 16):
                        tn = min(16, n - t0)
                        pairs = h_prev[:, 2 * t0 * E:2 * (t0 + tn) * E].rearrange(
                            "p (t c) -> p t c", c=2 * E)
                        nc.vector.tensor_add(
                            s_cur[:, t0 * E:(t0 + tn) * E].rearrange(
                                "p (t m) -> p t m", m=E),
                            pairs[:, :, 0:E], pairs[:, :, E:2 * E])

                for (g0, gn, kind) in units[lvl]:
                    pool = pool_ps1 if gn == 8 else pool_ps2
                    ps = pool.tile([128, gn * E], f32, tag=f"ps{gn}")
                    if kind == "sc":
                        boff = (slot_base + g0) * E
                        nc.tensor.matmul(
                            out=ps[:, :], lhsT=ones2[:, :],
                            rhs=bt[:, boff:boff + gn * E], start=True,
                            stop=False, skip_group_check=True)
                    for i in range(gn):
                        t = g0 + i
                        w, off = slot_chunk[slot_base + t]
                        nc.tensor.matmul(
                            out=ps[:, i * E:(i + 1) * E],
                            lhsT=w[:, off * 128:(off + 1) * 128],
                            rhs=s_cur[:, t * E:(t + 1) * E],
                            start=(kind == "dve" and i % 8 == 0), stop=True,
                            skip_group_check=True)
                    dst = h_new[:, g0 * E:(g0 + gn) * E]
                    if kind == "sc":
                        # alternate drain engine per bank: bias is already in
                        # PSUM, so DVE can relu via a two-op tensor_scalar
                        if lvl == 3 or ((slot_base + g0) // 8) % 2 == 0:
                            nc.scalar.activation(
                                out=dst, in_=ps[:, :],
                                func=mybir.ActivationFunctionType.Relu,
                                scale=inv)
                        else:
                            nc.vector.tensor_scalar(
                                dst, ps[:, :], inv, 0.0,
                                op0=mybir.AluOpType.mult,
                                op1=mybir.AluOpType.max)
                    else:
                        do = BPOS[slot_base + g0] * E
                        nc.vector.scalar_tensor_tensor(
                            dst, ps[:, :], inv, bbt[:, do:do + gn * E],
                            op0=mybir.AluOpType.mult, op1=mybir.AluOpType.max)

                h_prev = h_new
                slot_base += n

            nc.sync.dma_start(out[:], h_prev[:, :])

    nc.compile()
    return nc


def _get_nc():
    if "nc" not in _CACHE:
        _CACHE["nc"] = _build_nc()
    return _CACHE["nc"]


def _pack_core(c, node_ids, emb, bias_table):
    """Packed wt/s8/bi/bb streams for core c (half-trees rooted at heap
    nodes 15+2c and 16+2c). Returns (in_map, phi3[2,8,E,E])."""
    wtz = np.zeros((2, E, NSLOT, 128), dtype=np.float32)  # [half, j, slot, col]
    bi = np.empty((2, NSLOT, E), dtype=np.float32)
    phiv = np.zeros((2, NSLOT, E, E), dtype=np.float32)   # per-node offsets
    s8p = np.empty((2, E, S8_SLOTS, E), dtype=np.float32)
    level_base = {}
    base = 0
    for lvl in DEV_LEVELS:
        level_base[lvl] = base
        base += 2 ** lvl
    for q in range(2):
        g0 = 15 + 2 * c + q
        Wlv, blv = {}, {}
        for lvl in DEV_LEVELS:
            n = 2 ** lvl
            start = (g0 + 1) * n - 1
            ids = node_ids[start:start + n]
            Wlv[lvl] = emb[ids].reshape(n, E, E)
            blv[lvl] = bias_table[ids].astype(np.float32)
            woff = level_base[lvl]
            wtz[q, :, woff:woff + n, q * E:(q + 1) * E] = \
                (Wlv[lvl] * WSCALE).transpose(2, 0, 1)
        # phi recursion in stream order 7 -> 3 (children first)
        phi = {}
        for lvl in DEV_LEVELS:
            n = 2 ** lvl
            woff = level_base[lvl]
            if lvl == 7:
                g = np.zeros((n, E, E), dtype=np.float32)
            else:
                pc = phi[lvl + 1]
                g = np.einsum('nij,njk->nik', Wlv[lvl], pc[0::2] + pc[1::2])
            bsc = WSCALE * S8SCALE if lvl == 7 else WSCALE
            ph = np.zeros((n, E, E), dtype=np.float32)
            for t in range(n):
                if (woff + t) // 8 not in SC_BANKS:
                    ph[t] = g[t] - blv[lvl][t][None, :]
            phi[lvl] = ph
            bi[q, woff:woff + n, :] = blv[lvl] * bsc  # used by SC banks (g=0)
            phiv[q, woff:woff + n] = ph
        # leaf level folded on host: s8 slot t = relu(W_2t) + relu(W_2t+1)
        nleaf = 2 ** (HT_D - 1)
        start = (g0 + 1) * nleaf - 1
        ids = node_ids[start:start + nleaf]
        leaf = np.maximum(emb[ids].reshape(nleaf, E, E), 0.0)
        s8p[q] = (leaf[0::2] + leaf[1::2]).transpose(1, 0, 2) * S8SCALE
    # beta stream: [128 partitions(i per half), dve-slot, k]
    bbs = np.empty((2, E, len(DVE_SLOTS), E), dtype=np.float32)
    for t in DVE_SLOTS:
        p = BPOS[t]
        for q in range(2):
            bbs[q, :, p, :] = phiv[q, t]
    on2 = np.zeros((2, 128), dtype=np.float32)
    on2[0, 0:E] = 1.0
    on2[1, E:128] = 1.0
    phi3 = phiv[:, level_base[3]:level_base[3] + 8]  # [2, 8, E, E]
    return {
        "wt": np.ascontiguousarray(wtz.reshape(128, NSLOT * 128)).astype(ml_dtypes.float8_e4m3),
        "s8": np.ascontiguousarray(s8p.reshape(128, S8_SLOTS * E)).astype(ml_dtypes.float8_e4m3),
        "bi": np.ascontiguousarray(bi.reshape(2, NSLOT * E)).astype(ml_dtypes.bfloat16),
        "on2": on2.astype(ml_dtypes.bfloat16),
        **({"bb": np.ascontiguousarray(bbs.reshape(128, len(DVE_SLOTS) * E)).astype(ml_dtypes.float8_e4m3)} if DVE_SLOTS else {}),
    }, phi3


def kernel(node_ids, label, embedding, bias_table, proj_w, proj_b):
    from concourse.bass_utils import run_bass_kernel_spmd

    node_ids = np.asarray(node_ids).astype(np.int64)
    emb = np.ascontiguousarray(np.asarray(embedding, dtype=np.float32))
    bias_table = np.ascontiguousarray(np.asarray(bias_table, dtype=np.float32))
    proj_w = np.asarray(proj_w, dtype=np.float32)
    proj_b = np.asarray(proj_b, dtype=np.float32)
    label_i = int(np.asarray(label))

    nc = _get_nc()
    packs = [_pack_core(c, node_ids, emb, bias_table) for c in range(NCORES)]
    in_maps = [p[0] for p in packs]
    res = run_bass_kernel_spmd(nc, in_maps, core_ids=list(range(NCORES)))

    # device ships h° at global heap level 7 (nodes 127..254): h = h° - phi
    h = np.empty((128, E, E), dtype=np.float32)
    for c in range(NCORES):
        o = res.results[c]["out"].astype(np.float32)  # [128, 8*64]
        phi3 = packs[c][1]
        for q in range(2):
            g0 = 15 + 2 * c + q
            base = (g0 + 1) * OUT_SLOTS - 1 - 127
            for t in range(OUT_SLOTS):
                h[base + t] = o[q * E:(q + 1) * E, t * E:(t + 1) * E] \
                    - phi3[q, t]
    for lvl in range(6, -1, -1):
        nlv = 2 ** lvl
        start = nlv - 1
        ids = node_ids[start:start + nlv]
        W = emb[ids].reshape(nlv, E, E)
        b = bias_table[ids]
        s = h[0::2] + h[1::2]
        h = np.maximum(np.einsum('nij,njk->nik', W, s) + b[:, None, :], 0.0)

    root = h[0].reshape(-1)
    logits = root @ proj_w.T + proj_b
    m = logits.max()
    lse = m + np.log(np.exp(logits - m).sum())
    log_softmax = logits - lse
    loss = np.float32(-log_softmax[label_i])
    prediction = np.int64(np.argmax(logits))
    return prediction, loss
